# revision 26
# baseline (speedup 1.0000x reference)
"""Causal self-attention (B=2, L=2048, E=2048, H=16, HD=128) on 8 trn2 cores.

Sharding: core c = (b, g) with b = c // 4 (batch), g = c % 4 (head group of 4).
Each core computes QKV projection for its 4 heads on its batch, causal
attention with RoPE, and a partial output projection (its heads' slice of
w_proj rows). Host sums the 4 partial projections per batch.

All matmuls run in bf16 with fp32 PSUM accumulation (measured end-to-end
rel. error ~5e-3 vs the fp32 reference).

Key device-side structure (per core):
  - phase 1, per 512-wide l-chunk: q/k/v projections as K-accumulated
    matmuls; rope fused right behind each q/k chunk:
        rot = (q * cs) - perm_swap(q * ss)        [2 DVE muls + PE swap + sub]
    with cs/ss host-prebuilt [128, L] tables (softmax scale folded in).
  - phase 2: scores computed transposed (sT[j,i] = k_j . q_i) so P@V needs
    no transpose; softmax without max-subtraction (|s| <= ~10); denominator
    via all-ones matmul which also broadcasts Z across partitions; causal
    masking by skipping upper-triangle blocks + 4 static diagonal masks;
    software-pipelined with a 3-deep score-matmul lookahead.
  - phase 3: partial out-projection, [f, l] layout, one big DMA per f-tile.

Device layouts (per core):
  xt    [E=2048, L=2048] bf16   x[b].T  (e on rows)
  wqk   [E, 1024]        bf16   8 col-blocks: q-heads 0..3, k-heads 0..3,
                                head rows perm'd to (even|odd) order, transposed
  wv    [E, 512]         bf16   v weights, natural order, transposed
  wout  [512, E]         bf16   w_proj[:, g*512:(g+1)*512].T
  cs,ss [128, L]         bf16   rope cos / (-sin|+sin) tables * 128**-0.25
  masks [128, 4*512]     bf16   causal diagonal-block masks
  ones  [128, 128]       bf16   all-ones (softmax denominator broadcast-sum)
  perm  [128, 128]       bf16   half-swap permutation (rope pair partner)
Output:
  out   [E, L] fp32  (partial projection, transposed; host adds + transposes)
"""

from contextlib import ExitStack

import numpy as np
import ml_dtypes

import concourse.bass as bass
import concourse.mybir as mybir
import concourse.tile as tile
from concourse import bacc
from concourse.bass_utils import run_bass_kernel_spmd

BF16 = ml_dtypes.bfloat16
B, L, E, H, HD = 2, 2048, 2048, 16, 128
G = 4            # head groups (cores per batch)
HPG = H // G     # heads per group = 4
NCORES = 8
NE = E // 128    # 16 e-chunks
NLC = L // 512   # 4 l-chunks of 512
NLT = L // 128   # 16 l-tiles of 128
SCALE = float(128.0 ** -0.25)   # per-operand score scale (q and k each)

FP32 = mybir.dt.float32
BF = mybir.dt.bfloat16


def build_nc():
    nc = bacc.Bacc(
        "TRN2",
        target_bir_lowering=False,
        debug=False,
        enable_asserts=False,
        num_devices=NCORES,
    )
    d = {}
    d["xt"] = nc.dram_tensor("xt", [E, L], BF, kind="ExternalInput").ap()
    d["wqk"] = nc.dram_tensor("wqk", [E, 2 * HPG * 128], BF, kind="ExternalInput").ap()
    d["wv"] = nc.dram_tensor("wv", [E, HPG * 128], BF, kind="ExternalInput").ap()
    d["wout"] = nc.dram_tensor("wout", [HPG * 128, E], BF, kind="ExternalInput").ap()
    d["cs"] = nc.dram_tensor("cs", [128, L], BF, kind="ExternalInput").ap()
    d["ss"] = nc.dram_tensor("ss", [128, L], BF, kind="ExternalInput").ap()
    d["masks"] = nc.dram_tensor("masks", [128, 4 * 512], BF, kind="ExternalInput").ap()
    d["ones"] = nc.dram_tensor("ones", [128, 128], BF, kind="ExternalInput").ap()
    d["perm"] = nc.dram_tensor("perm", [128, 128], BF, kind="ExternalInput").ap()
    d["out"] = nc.dram_tensor("out", [E, L], mybir.dt.float16,
                              kind="ExternalOutput").ap()

    with tile.TileContext(nc) as tc:
        build_kernel(tc, d)
    nc.compile()
    return nc


def build_kernel(tc, d):
    nc = tc.nc
    EXP = mybir.ActivationFunctionType.Exp

    with ExitStack() as ctx:
        const = ctx.enter_context(tc.tile_pool(name="const", bufs=1))
        qkres = ctx.enter_context(tc.tile_pool(name="qkres", bufs=1))
        vres = ctx.enter_context(tc.tile_pool(name="vres", bufs=1))
        yres = ctx.enter_context(tc.tile_pool(name="yres", bufs=1))
        xs = ctx.enter_context(tc.tile_pool(name="xs", bufs=20))
        atile = ctx.enter_context(tc.tile_pool(name="atile", bufs=6))
        pexp = ctx.enter_context(tc.tile_pool(name="pexp", bufs=6))
        zpool = ctx.enter_context(tc.tile_pool(name="zpool", bufs=3))
        outst = ctx.enter_context(tc.tile_pool(name="outst", bufs=2))

        # ---- constants / weights ----
        # wqk chunks are emitted interleaved with the first l-chunk's xt tiles
        # inside the phase-1 loop (so queue arrival order matches the PE's
        # consumption order); allocate the tile here.
        wqk_sb = const.tile([128, NE, 2 * HPG * 128], BF, name="wqk_sb", tag="wqk_sb")
        wqk_r = d["wqk"].rearrange("(ec p) f -> p ec f", p=128)
        # odd wqk chunks go at the head of the gpsimd queue (consumed at
        # ~1.7us/chunk by the interleaved first qk-pass)
        for e in range(1, NE, 2):
            nc.gpsimd.dma_start(out=wqk_sb[:, e, :], in_=wqk_r[:, e, :])
        cs_sb = const.tile([128, L], BF, name="cs_sb", tag="cs_sb")
        nc.gpsimd.dma_start(out=cs_sb, in_=d["cs"])
        ss_sb = const.tile([128, L], BF, name="ss_sb", tag="ss_sb")
        nc.gpsimd.dma_start(out=ss_sb, in_=d["ss"])
        perm_sb = const.tile([128, 128], BF, name="perm_sb", tag="perm_sb")
        nc.gpsimd.dma_start(out=perm_sb, in_=d["perm"])
        wv_sb = const.tile([128, NE, HPG * 128], BF, name="wv_sb", tag="wv_sb")
        wv_r = d["wv"].rearrange("(ec p) f -> p ec f", p=128)
        for e in range(NE):
            nc.scalar.dma_start(out=wv_sb[:, e, :], in_=wv_r[:, e, :])
        # masks/ones/wout are needed only from the attention phase on; their
        # DMAs are emitted after phase 1 so they don't contend with the
        # startup-critical weight/x loads. Tiles allocated here.
        masks_sb = const.tile([128, 4, 512], BF, name="masks_sb", tag="masks_sb")
        ones_sb = const.tile([128, 128], BF, name="ones_sb", tag="ones_sb")
        wout_sb = const.tile([128, HPG, E], BF, name="wout_sb", tag="wout_sb")

        # ---- residents ----
        q_sb = [qkres.tile([128, L], BF, name=f"q_sb{h}", tag=f"q_sb{h}")
                for h in range(HPG)]
        k_sb = [qkres.tile([128, L], BF, name=f"k_sb{h}", tag=f"k_sb{h}")
                for h in range(HPG)]
        v_sb = vres.tile([128, NLT, HPG * 128], BF, name="v_sb", tag="v_sb")
        y_sb = [yres.tile([128, L], BF, name=f"y_sb{h}", tag=f"y_sb{h}")
                for h in range(HPG)]

        # ================= phase 1: QKV projection + fused rope ============
        with tc.tile_pool(name="psum1", bufs=1, space="PSUM") as ps1:

            def acc_tile(nm):
                return ps1.tile([128, 512], FP32, name=nm, tag="pacc", bufs=8)

            for lc in range(NLC):
                ls_lo = lc * 512
                cs_lc = cs_sb[:, ls_lo:ls_lo + 512]
                ss_lc = ss_sb[:, ls_lo:ls_lo + 512]

                xt_t = []
                for e in range(NE):
                    t = xs.tile([128, 512], BF, name=f"xt_{lc}_{e}", tag="xt")
                    if lc == 0:
                        # x tiles interleaved with even weight chunks on the
                        # hw (sync) queue; odd chunks stream on gpsimd so the
                        # first qk-pass can consume e=0..15 without waiting
                        # for the full weight load
                        nc.sync.dma_start(
                            out=t,
                            in_=d["xt"][e * 128:(e + 1) * 128, ls_lo:ls_lo + 512])
                        if e % 2 == 0:
                            nc.sync.dma_start(out=wqk_sb[:, e, :],
                                              in_=wqk_r[:, e, :])
                    else:
                        nc.sync.dma_start(
                            out=t,
                            in_=d["xt"][e * 128:(e + 1) * 128, ls_lo:ls_lo + 512])
                    xt_t.append(t)

                def qk_pass(halves):
                    # projection matmuls for the given f-block halves
                    # (0 = q heads, 1 = k heads); passing both interleaves
                    # them per e-chunk, which halves the weight-chunk arrival
                    # rate the PE needs (used for the DMA-bound first l-chunk)
                    acc = [acc_tile(f"p{half}_{lc}_{h}")
                           for half in halves for h in range(HPG)]
                    for e in range(NE):
                        for i, half in enumerate(halves):
                            for h in range(HPG):
                                fb = half * HPG + h
                                nc.tensor.matmul(
                                    acc[i * HPG + h],
                                    lhsT=wqk_sb[:, e, fb * 128:(fb + 1) * 128],
                                    rhs=xt_t[e],
                                    start=(e == 0), stop=(e == NE - 1))
                    return acc

                def rope_a(acc, which):
                    # a = q*ss (bf16), dst-slice = q*cs ; releases acc
                    a_t = []
                    for h in range(HPG):
                        a = atile.tile([128, 512], BF,
                                       name=f"a_{which}{h}_{lc}", tag="a")
                        nc.vector.tensor_mul(out=a, in0=acc[h], in1=ss_lc)
                        dst = (q_sb if which == "q" else k_sb)[h]
                        nc.vector.tensor_mul(
                            out=dst[:, ls_lo:ls_lo + 512], in0=acc[h], in1=cs_lc)
                        a_t.append(a)
                    return a_t

                def rope_b(a_t, which):
                    # dst -= perm_swap(a)
                    for h in range(HPG):
                        psw = acc_tile(f"psw_{which}{h}_{lc}")
                        nc.tensor.matmul(psw, lhsT=perm_sb, rhs=a_t[h],
                                         start=True, stop=True)
                        dst = (q_sb if which == "q" else k_sb)[h]
                        sl = dst[:, ls_lo:ls_lo + 512]
                        nc.vector.tensor_sub(out=sl, in0=sl, in1=psw)

                def v_pass():
                    # v pass (x tiles stationary -> natural [l, d] layout)
                    for ls in range(4):
                        lt = lc * 4 + ls
                        accv = acc_tile(f"pv_{lt}")
                        for e in range(NE):
                            nc.tensor.matmul(
                                accv,
                                lhsT=xt_t[e][:, ls * 128:(ls + 1) * 128],
                                rhs=wv_sb[:, e, :],
                                start=(e == 0), stop=(e == NE - 1))
                        nc.scalar.copy(out=v_sb[:, lt, :], in_=accv)

                if lc == 0:
                    acc8 = qk_pass((0, 1))
                    accq, acck = acc8[:HPG], acc8[HPG:]
                    aq = rope_a(accq, "q")
                    ak = rope_a(acck, "k")
                    rope_b(aq, "q")
                    v_pass()
                    rope_b(ak, "k")
                elif lc == NLC - 1:
                    # last chunk: finalize k before the v pass so attention's
                    # first score matmuls aren't gated on the v matmuls
                    accq = qk_pass((0,))
                    aq = rope_a(accq, "q")
                    acck = qk_pass((1,))
                    rope_b(aq, "q")
                    ak = rope_a(acck, "k")
                    rope_b(ak, "k")
                    v_pass()
                else:
                    accq = qk_pass((0,))
                    aq = rope_a(accq, "q")
                    acck = qk_pass((1,))
                    rope_b(aq, "q")
                    ak = rope_a(acck, "k")
                    v_pass()
                    rope_b(ak, "k")

        # ======== phase 2+3: causal attention with interleaved projection ==
        # jobs are ic-major: once all 4 heads finished l-chunk ic, that
        # chunk's output projection is emitted immediately — it fills
        # attention pipeline bubbles and spreads the output DMA.
        nc.gpsimd.dma_start(
            out=masks_sb, in_=d["masks"].rearrange("p (r f) -> p r f", r=4))
        nc.gpsimd.dma_start(out=ones_sb, in_=d["ones"])
        nc.gpsimd.dma_start(
            out=wout_sb, in_=d["wout"].rearrange("(h p) f -> p h f", p=128))
        with tc.tile_pool(name="psum2", bufs=1, space="PSUM") as ps2:
            jobs = [(h, ic) for ic in range(NLC) for h in range(HPG)]
            steps = [(ji, jb)
                     for ji, (_h, ic) in enumerate(jobs)
                     for jb in range(4 * ic + 4)]
            LA = 3
            pss_map = {}
            zy = {}

            def emit_s(ji, jb):
                h, ic = jobs[ji]
                # diagonal blocks (r >= 1) have no valid columns below
                # f = 128*r: compute only the valid column range
                r = jb - 4 * ic
                lo = r * 128 if r > 0 else 0
                t = ps2.tile([128, 512], FP32, name=f"pss_{ji}_{jb}",
                             tag="pss", bufs=4)
                nc.tensor.matmul(
                    t[:, lo:],
                    lhsT=k_sb[h][:, jb * 128:(jb + 1) * 128],
                    rhs=q_sb[h][:, ic * 512 + lo:(ic + 1) * 512],
                    start=True, stop=True)
                pss_map[(ji, jb)] = t

            ptr = 0
            for idx, (ji, jb) in enumerate(steps):
                while ptr < len(steps) and ptr <= idx + LA:
                    emit_s(*steps[ptr])
                    ptr += 1
                h, ic = jobs[ji]
                njb = 4 * ic + 4
                if jb == 0:
                    zy[ji] = (
                        ps2.tile([128, 512], FP32, name=f"psz_{ji}",
                                 tag="pzy", bufs=4),
                        ps2.tile([128, 512], FP32, name=f"psy_{ji}",
                                 tag="pzy", bufs=4),
                    )
                psz, psy = zy[ji]
                pss = pss_map.pop((ji, jb))
                r = jb - 4 * ic
                lo = r * 128 if r > 0 else 0
                pt = pexp.tile([128, 512], BF, name=f"pt_{ji}_{jb}", tag="pexp")
                nc.scalar.activation(out=pt[:, lo:], in_=pss[:, lo:], func=EXP)
                if r >= 0:
                    # diagonal block: only the first 128 columns of the valid
                    # range hold the per-element triangle; the rest are all-1
                    nc.vector.tensor_mul(
                        out=pt[:, lo:lo + 128], in0=pt[:, lo:lo + 128],
                        in1=masks_sb[:, r, lo:lo + 128])
                nc.tensor.matmul(psz[:, lo:], lhsT=ones_sb, rhs=pt[:, lo:],
                                 start=(jb == 0), stop=(jb == njb - 1))
                nc.tensor.matmul(psy[:, lo:],
                                 lhsT=v_sb[:, jb, h * 128:(h + 1) * 128],
                                 rhs=pt[:, lo:],
                                 start=(jb == 0), stop=(jb == njb - 1))
                if jb == njb - 1:
                    zv = zpool.tile([128, 512], FP32, name=f"zinv_{ji}",
                                    tag="zinv")
                    nc.vector.reciprocal_approx_fast(out=zv, in_=psz)
                    nc.vector.tensor_mul(
                        out=y_sb[h][:, ic * 512:(ic + 1) * 512],
                        in0=psy, in1=zv)
                    if h == HPG - 1:
                        # all heads done for this l-chunk: emit its projection
                        lc = ic
                        for ft in range(NE):
                            po = ps2.tile([128, 512], FP32,
                                          name=f"po_{ft}_{lc}", tag="pzy",
                                          bufs=4)
                            for hh in range(HPG):
                                nc.tensor.matmul(
                                    po,
                                    lhsT=wout_sb[:, hh,
                                                 ft * 128:(ft + 1) * 128],
                                    rhs=y_sb[hh][:, lc * 512:(lc + 1) * 512],
                                    start=(hh == 0), stop=(hh == HPG - 1))
                            ot = outst.tile([128, 512], mybir.dt.float16,
                                            name=f"ot_{ft}_{lc}", tag="ot",
                                            bufs=6)
                            if ft % 2 == 0:
                                nc.vector.tensor_copy(out=ot, in_=po)
                            else:
                                nc.scalar.copy(out=ot, in_=po)
                            eng = (nc.sync, nc.gpsimd, nc.sync,
                                   nc.scalar)[ft % 4]
                            eng.dma_start(
                                out=d["out"][ft * 128:(ft + 1) * 128,
                                             lc * 512:(lc + 1) * 512],
                                in_=ot)


# ------------------------------------------------------------------ host side

_PERM_IDX = np.concatenate([np.arange(0, 128, 2), np.arange(1, 128, 2)])


def prep_in_maps(x, rope, w_attn, w_proj):
    x = np.asarray(x, np.float32)
    rope = np.asarray(rope, np.float32)
    w_attn = np.asarray(w_attn, np.float32)
    w_proj = np.asarray(w_proj, np.float32)

    sin = rope[:, :, 0]                      # [L, 64]
    cos = rope[:, :, 1]
    cs = (np.concatenate([cos.T, cos.T], 0) * SCALE).astype(BF16)   # [128, L]
    ss = (np.concatenate([-sin.T, sin.T], 0) * SCALE).astype(BF16)

    p = np.arange(128)[:, None]
    f = np.arange(512)[None, :]
    masks = np.zeros((128, 4, 512), np.float32)
    for r in range(4):
        masks[:, r, :] = (r * 128 + p <= f).astype(np.float32)
    masks = masks.reshape(128, 4 * 512).astype(BF16)

    ones = np.ones((128, 128), np.float32).astype(BF16)
    perm = np.zeros((128, 128), np.float32)
    perm[(np.arange(128) + 64) % 128, np.arange(128)] = 1.0
    perm = perm.astype(BF16)

    xt_b = [np.ascontiguousarray(x[b].T).astype(BF16) for b in range(B)]

    wqk_g, wv_g, wout_g = {}, {}, {}
    for g in range(G):
        heads = [g * HPG + hl for hl in range(HPG)]
        wq = [np.ascontiguousarray(
                 w_attn[h * 128:(h + 1) * 128, :][_PERM_IDX, :].T) for h in heads]
        wk = [np.ascontiguousarray(
                 w_attn[E + h * 128:E + (h + 1) * 128, :][_PERM_IDX, :].T)
              for h in heads]
        wqk_g[g] = np.concatenate(wq + wk, axis=1).astype(BF16)        # [E, 1024]
        wv_g[g] = np.concatenate(
            [w_attn[2 * E + h * 128:2 * E + (h + 1) * 128, :].T for h in heads],
            axis=1).astype(BF16)                                        # [E, 512]
        wout_g[g] = np.ascontiguousarray(
            w_proj[:, g * 512:(g + 1) * 512].T).astype(BF16)            # [512, E]

    in_maps = []
    for c in range(NCORES):
        b, g = divmod(c, G)
        in_maps.append({
            "xt": xt_b[b],
            "wqk": wqk_g[g],
            "wv": wv_g[g],
            "wout": wout_g[g],
            "cs": cs,
            "ss": ss,
            "masks": masks,
            "ones": ones,
            "perm": perm,
        })
    return in_maps


def assemble_output(results):
    out = np.zeros((B, L, E), np.float32)
    for c in range(NCORES):
        b, g = divmod(c, G)
        out[b] += results[c]["out"].T
    return out


_NC = None


def get_nc():
    global _NC
    if _NC is None:
        _NC = build_nc()
    return _NC


def run(x, rope, w_attn, w_proj, trace=False, tmpdir=None):
    nc = get_nc()
    in_maps = prep_in_maps(x, rope, w_attn, w_proj)
    kwargs = {}
    if trace:
        import sys
        import types
        from concourse import bass_utils as _bu
        try:
            from trn_agent_boot.trn_boot import _ntff_profile_via_ctypes
            hook = _ntff_profile_via_ctypes("/opt/axon/libaxon_pjrt.so")
            mod = types.ModuleType("antenv.axon_hooks")
            mod.get_axon_ntff_profile_hook = lambda: hook
            sys.modules["antenv.axon_hooks"] = mod
            _bu.upload_artifacts = lambda dd: dd
        except Exception as e:  # pragma: no cover
            print("trace hook unavailable:", e)
        kwargs = dict(trace=True, tmpdir=tmpdir)
    res = run_bass_kernel_spmd(nc, in_maps, core_ids=list(range(NCORES)), **kwargs)
    return assemble_output(res.results), res


def kernel(x, rope, w_attn, w_proj):
    out, _ = run(x, rope, w_attn, w_proj, trace=False)
    return out


# revision 27
# speedup vs baseline: 1.0011x; 1.0011x over previous
"""Causal self-attention (B=2, L=2048, E=2048, H=16, HD=128) on 8 trn2 cores.

Sharding: core c = (b, g) with b = c // 4 (batch), g = c % 4 (head group of 4).
Each core computes QKV projection for its 4 heads on its batch, causal
attention with RoPE, and a partial output projection (its heads' slice of
w_proj rows). Host sums the 4 partial projections per batch.

All matmuls run in bf16 with fp32 PSUM accumulation (measured end-to-end
rel. error ~5e-3 vs the fp32 reference).

Key device-side structure (per core):
  - phase 1, per 512-wide l-chunk: q/k/v projections as K-accumulated
    matmuls; rope fused right behind each q/k chunk:
        rot = (q * cs) - perm_swap(q * ss)        [2 DVE muls + PE swap + sub]
    with cs/ss host-prebuilt [128, L] tables (softmax scale folded in).
  - phase 2: scores computed transposed (sT[j,i] = k_j . q_i) so P@V needs
    no transpose; softmax without max-subtraction (|s| <= ~10); denominator
    via all-ones matmul which also broadcasts Z across partitions; causal
    masking by skipping upper-triangle blocks + 4 static diagonal masks;
    software-pipelined with a 3-deep score-matmul lookahead.
  - phase 3: partial out-projection, [f, l] layout, one big DMA per f-tile.

Device layouts (per core):
  xt    [E=2048, L=2048] bf16   x[b].T  (e on rows)
  wqk   [E, 1024]        bf16   8 col-blocks: q-heads 0..3, k-heads 0..3,
                                head rows perm'd to (even|odd) order, transposed
  wv    [E, 512]         bf16   v weights, natural order, transposed
  wout  [512, E]         bf16   w_proj[:, g*512:(g+1)*512].T
  cs,ss [128, L]         bf16   rope cos / (-sin|+sin) tables * 128**-0.25
  masks [128, 4*512]     bf16   causal diagonal-block masks
  ones  [128, 128]       bf16   all-ones (softmax denominator broadcast-sum)
  perm  [128, 128]       bf16   half-swap permutation (rope pair partner)
Output:
  out   [E, L] fp32  (partial projection, transposed; host adds + transposes)
"""

from contextlib import ExitStack

import numpy as np
import ml_dtypes

import concourse.bass as bass
import concourse.mybir as mybir
import concourse.tile as tile
from concourse import bacc
from concourse.bass_utils import run_bass_kernel_spmd

BF16 = ml_dtypes.bfloat16
B, L, E, H, HD = 2, 2048, 2048, 16, 128
G = 4            # head groups (cores per batch)
HPG = H // G     # heads per group = 4
NCORES = 8
NE = E // 128    # 16 e-chunks
NLC = L // 512   # 4 l-chunks of 512
NLT = L // 128   # 16 l-tiles of 128
SCALE = float(128.0 ** -0.25)   # per-operand score scale (q and k each)

FP32 = mybir.dt.float32
BF = mybir.dt.bfloat16


def build_nc():
    nc = bacc.Bacc(
        "TRN2",
        target_bir_lowering=False,
        debug=False,
        enable_asserts=False,
        num_devices=NCORES,
    )
    d = {}
    d["xt"] = nc.dram_tensor("xt", [E, L], BF, kind="ExternalInput").ap()
    d["wqk"] = nc.dram_tensor("wqk", [E, 2 * HPG * 128], BF, kind="ExternalInput").ap()
    d["wv"] = nc.dram_tensor("wv", [E, HPG * 128], BF, kind="ExternalInput").ap()
    d["wout"] = nc.dram_tensor("wout", [HPG * 128, E], BF, kind="ExternalInput").ap()
    d["cs"] = nc.dram_tensor("cs", [128, L], BF, kind="ExternalInput").ap()
    d["ss"] = nc.dram_tensor("ss", [128, L], BF, kind="ExternalInput").ap()
    d["masks"] = nc.dram_tensor("masks", [128, 4 * 512], BF, kind="ExternalInput").ap()
    d["ones"] = nc.dram_tensor("ones", [128, 128], BF, kind="ExternalInput").ap()
    d["perm"] = nc.dram_tensor("perm", [128, 128], BF, kind="ExternalInput").ap()
    d["out"] = nc.dram_tensor("out", [E, L], mybir.dt.float16,
                              kind="ExternalOutput").ap()

    with tile.TileContext(nc) as tc:
        build_kernel(tc, d)
    nc.compile()
    return nc


def build_kernel(tc, d):
    nc = tc.nc
    EXP = mybir.ActivationFunctionType.Exp

    with ExitStack() as ctx:
        const = ctx.enter_context(tc.tile_pool(name="const", bufs=1))
        qkres = ctx.enter_context(tc.tile_pool(name="qkres", bufs=1))
        vres = ctx.enter_context(tc.tile_pool(name="vres", bufs=1))
        yres = ctx.enter_context(tc.tile_pool(name="yres", bufs=1))
        xs = ctx.enter_context(tc.tile_pool(name="xs", bufs=24))
        atile = ctx.enter_context(tc.tile_pool(name="atile", bufs=8))
        pexp = ctx.enter_context(tc.tile_pool(name="pexp", bufs=10))
        zpool = ctx.enter_context(tc.tile_pool(name="zpool", bufs=3))
        outst = ctx.enter_context(tc.tile_pool(name="outst", bufs=2))

        # ---- constants / weights ----
        # wqk chunks are emitted interleaved with the first l-chunk's xt tiles
        # inside the phase-1 loop (so queue arrival order matches the PE's
        # consumption order); allocate the tile here.
        wqk_sb = const.tile([128, NE, 2 * HPG * 128], BF, name="wqk_sb", tag="wqk_sb")
        wqk_r = d["wqk"].rearrange("(ec p) f -> p ec f", p=128)
        # odd wqk chunks go at the head of the gpsimd queue (consumed at
        # ~1.7us/chunk by the interleaved first qk-pass)
        for e in range(1, NE, 2):
            nc.gpsimd.dma_start(out=wqk_sb[:, e, :], in_=wqk_r[:, e, :])
        cs_sb = const.tile([128, L], BF, name="cs_sb", tag="cs_sb")
        nc.gpsimd.dma_start(out=cs_sb, in_=d["cs"])
        ss_sb = const.tile([128, L], BF, name="ss_sb", tag="ss_sb")
        nc.gpsimd.dma_start(out=ss_sb, in_=d["ss"])
        perm_sb = const.tile([128, 128], BF, name="perm_sb", tag="perm_sb")
        nc.gpsimd.dma_start(out=perm_sb, in_=d["perm"])
        wv_sb = const.tile([128, NE, HPG * 128], BF, name="wv_sb", tag="wv_sb")
        wv_r = d["wv"].rearrange("(ec p) f -> p ec f", p=128)
        for e in range(NE):
            nc.scalar.dma_start(out=wv_sb[:, e, :], in_=wv_r[:, e, :])
        # masks/ones/wout are needed only from the attention phase on; their
        # DMAs are emitted after phase 1 so they don't contend with the
        # startup-critical weight/x loads. Tiles allocated here.
        masks_sb = const.tile([128, 4, 512], BF, name="masks_sb", tag="masks_sb")
        ones_sb = const.tile([128, 128], BF, name="ones_sb", tag="ones_sb")
        wout_sb = const.tile([128, HPG, E], BF, name="wout_sb", tag="wout_sb")

        # ---- residents ----
        q_sb = [qkres.tile([128, L], BF, name=f"q_sb{h}", tag=f"q_sb{h}")
                for h in range(HPG)]
        k_sb = [qkres.tile([128, L], BF, name=f"k_sb{h}", tag=f"k_sb{h}")
                for h in range(HPG)]
        v_sb = vres.tile([128, NLT, HPG * 128], BF, name="v_sb", tag="v_sb")
        y_sb = [yres.tile([128, L], BF, name=f"y_sb{h}", tag=f"y_sb{h}")
                for h in range(HPG)]

        # ================= phase 1: QKV projection + fused rope ============
        with tc.tile_pool(name="psum1", bufs=1, space="PSUM") as ps1:

            def acc_tile(nm):
                return ps1.tile([128, 512], FP32, name=nm, tag="pacc", bufs=8)

            for lc in range(NLC):
                ls_lo = lc * 512
                cs_lc = cs_sb[:, ls_lo:ls_lo + 512]
                ss_lc = ss_sb[:, ls_lo:ls_lo + 512]

                xt_t = []
                for e in range(NE):
                    t = xs.tile([128, 512], BF, name=f"xt_{lc}_{e}", tag="xt")
                    if lc == 0:
                        # x tiles interleaved with even weight chunks on the
                        # hw (sync) queue; odd chunks stream on gpsimd so the
                        # first qk-pass can consume e=0..15 without waiting
                        # for the full weight load
                        nc.sync.dma_start(
                            out=t,
                            in_=d["xt"][e * 128:(e + 1) * 128, ls_lo:ls_lo + 512])
                        if e % 2 == 0:
                            nc.sync.dma_start(out=wqk_sb[:, e, :],
                                              in_=wqk_r[:, e, :])
                    else:
                        nc.sync.dma_start(
                            out=t,
                            in_=d["xt"][e * 128:(e + 1) * 128, ls_lo:ls_lo + 512])
                    xt_t.append(t)

                def qk_pass(halves):
                    # projection matmuls for the given f-block halves
                    # (0 = q heads, 1 = k heads); passing both interleaves
                    # them per e-chunk, which halves the weight-chunk arrival
                    # rate the PE needs (used for the DMA-bound first l-chunk)
                    acc = [acc_tile(f"p{half}_{lc}_{h}")
                           for half in halves for h in range(HPG)]
                    for e in range(NE):
                        for i, half in enumerate(halves):
                            for h in range(HPG):
                                fb = half * HPG + h
                                nc.tensor.matmul(
                                    acc[i * HPG + h],
                                    lhsT=wqk_sb[:, e, fb * 128:(fb + 1) * 128],
                                    rhs=xt_t[e],
                                    start=(e == 0), stop=(e == NE - 1))
                    return acc

                def rope_a(acc, which):
                    # a = q*ss (bf16), dst-slice = q*cs ; releases acc
                    a_t = []
                    for h in range(HPG):
                        a = atile.tile([128, 512], BF,
                                       name=f"a_{which}{h}_{lc}", tag="a")
                        nc.vector.tensor_mul(out=a, in0=acc[h], in1=ss_lc)
                        dst = (q_sb if which == "q" else k_sb)[h]
                        nc.vector.tensor_mul(
                            out=dst[:, ls_lo:ls_lo + 512], in0=acc[h], in1=cs_lc)
                        a_t.append(a)
                    return a_t

                def rope_b(a_t, which):
                    # dst -= perm_swap(a)
                    for h in range(HPG):
                        psw = acc_tile(f"psw_{which}{h}_{lc}")
                        nc.tensor.matmul(psw, lhsT=perm_sb, rhs=a_t[h],
                                         start=True, stop=True)
                        dst = (q_sb if which == "q" else k_sb)[h]
                        sl = dst[:, ls_lo:ls_lo + 512]
                        nc.vector.tensor_sub(out=sl, in0=sl, in1=psw)

                def v_pass():
                    # v pass (x tiles stationary -> natural [l, d] layout)
                    for ls in range(4):
                        lt = lc * 4 + ls
                        accv = acc_tile(f"pv_{lt}")
                        for e in range(NE):
                            nc.tensor.matmul(
                                accv,
                                lhsT=xt_t[e][:, ls * 128:(ls + 1) * 128],
                                rhs=wv_sb[:, e, :],
                                start=(e == 0), stop=(e == NE - 1))
                        nc.scalar.copy(out=v_sb[:, lt, :], in_=accv)

                if lc == 0:
                    acc8 = qk_pass((0, 1))
                    accq, acck = acc8[:HPG], acc8[HPG:]
                    aq = rope_a(accq, "q")
                    ak = rope_a(acck, "k")
                    rope_b(aq, "q")
                    v_pass()
                    rope_b(ak, "k")
                elif lc == NLC - 1:
                    # last chunk: finalize k before the v pass so attention's
                    # first score matmuls aren't gated on the v matmuls
                    accq = qk_pass((0,))
                    aq = rope_a(accq, "q")
                    acck = qk_pass((1,))
                    rope_b(aq, "q")
                    ak = rope_a(acck, "k")
                    rope_b(ak, "k")
                    v_pass()
                else:
                    accq = qk_pass((0,))
                    aq = rope_a(accq, "q")
                    acck = qk_pass((1,))
                    rope_b(aq, "q")
                    ak = rope_a(acck, "k")
                    v_pass()
                    rope_b(ak, "k")

        # ======== phase 2+3: causal attention with interleaved projection ==
        # jobs are ic-major: once all 4 heads finished l-chunk ic, that
        # chunk's output projection is emitted immediately — it fills
        # attention pipeline bubbles and spreads the output DMA.
        nc.gpsimd.dma_start(
            out=masks_sb, in_=d["masks"].rearrange("p (r f) -> p r f", r=4))
        nc.gpsimd.dma_start(out=ones_sb, in_=d["ones"])
        nc.gpsimd.dma_start(
            out=wout_sb, in_=d["wout"].rearrange("(h p) f -> p h f", p=128))
        with tc.tile_pool(name="psum2", bufs=1, space="PSUM") as ps2:
            jobs = [(h, ic) for ic in range(NLC) for h in range(HPG)]
            steps = [(ji, jb)
                     for ji, (_h, ic) in enumerate(jobs)
                     for jb in range(4 * ic + 4)]
            LA = 3
            pss_map = {}
            zy = {}

            def emit_s(ji, jb):
                h, ic = jobs[ji]
                # diagonal blocks (r >= 1) have no valid columns below
                # f = 128*r: compute only the valid column range
                r = jb - 4 * ic
                lo = r * 128 if r > 0 else 0
                t = ps2.tile([128, 512], FP32, name=f"pss_{ji}_{jb}",
                             tag="pss", bufs=4)
                nc.tensor.matmul(
                    t[:, lo:],
                    lhsT=k_sb[h][:, jb * 128:(jb + 1) * 128],
                    rhs=q_sb[h][:, ic * 512 + lo:(ic + 1) * 512],
                    start=True, stop=True)
                pss_map[(ji, jb)] = t

            ptr = 0
            for idx, (ji, jb) in enumerate(steps):
                while ptr < len(steps) and ptr <= idx + LA:
                    emit_s(*steps[ptr])
                    ptr += 1
                h, ic = jobs[ji]
                njb = 4 * ic + 4
                if jb == 0:
                    zy[ji] = (
                        ps2.tile([128, 512], FP32, name=f"psz_{ji}",
                                 tag="pzy", bufs=4),
                        ps2.tile([128, 512], FP32, name=f"psy_{ji}",
                                 tag="pzy", bufs=4),
                    )
                psz, psy = zy[ji]
                pss = pss_map.pop((ji, jb))
                r = jb - 4 * ic
                lo = r * 128 if r > 0 else 0
                pt = pexp.tile([128, 512], BF, name=f"pt_{ji}_{jb}", tag="pexp")
                nc.scalar.activation(out=pt[:, lo:], in_=pss[:, lo:], func=EXP)
                if r >= 0:
                    # diagonal block: only the first 128 columns of the valid
                    # range hold the per-element triangle; the rest are all-1
                    nc.vector.tensor_mul(
                        out=pt[:, lo:lo + 128], in0=pt[:, lo:lo + 128],
                        in1=masks_sb[:, r, lo:lo + 128])
                nc.tensor.matmul(psz[:, lo:], lhsT=ones_sb, rhs=pt[:, lo:],
                                 start=(jb == 0), stop=(jb == njb - 1))
                nc.tensor.matmul(psy[:, lo:],
                                 lhsT=v_sb[:, jb, h * 128:(h + 1) * 128],
                                 rhs=pt[:, lo:],
                                 start=(jb == 0), stop=(jb == njb - 1))
                if jb == njb - 1:
                    zv = zpool.tile([128, 512], FP32, name=f"zinv_{ji}",
                                    tag="zinv")
                    nc.vector.reciprocal_approx_fast(out=zv, in_=psz)
                    nc.vector.tensor_mul(
                        out=y_sb[h][:, ic * 512:(ic + 1) * 512],
                        in0=psy, in1=zv)
                    if h == HPG - 1:
                        # all heads done for this l-chunk: emit its projection
                        lc = ic
                        for ft in range(NE):
                            po = ps2.tile([128, 512], FP32,
                                          name=f"po_{ft}_{lc}", tag="pzy",
                                          bufs=4)
                            for hh in range(HPG):
                                nc.tensor.matmul(
                                    po,
                                    lhsT=wout_sb[:, hh,
                                                 ft * 128:(ft + 1) * 128],
                                    rhs=y_sb[hh][:, lc * 512:(lc + 1) * 512],
                                    start=(hh == 0), stop=(hh == HPG - 1))
                            ot = outst.tile([128, 512], mybir.dt.float16,
                                            name=f"ot_{ft}_{lc}", tag="ot",
                                            bufs=6)
                            if ft % 2 == 0:
                                nc.vector.tensor_copy(out=ot, in_=po)
                            else:
                                nc.scalar.copy(out=ot, in_=po)
                            eng = (nc.sync, nc.gpsimd, nc.sync,
                                   nc.scalar)[ft % 4]
                            eng.dma_start(
                                out=d["out"][ft * 128:(ft + 1) * 128,
                                             lc * 512:(lc + 1) * 512],
                                in_=ot)


# ------------------------------------------------------------------ host side

_PERM_IDX = np.concatenate([np.arange(0, 128, 2), np.arange(1, 128, 2)])


def prep_in_maps(x, rope, w_attn, w_proj):
    x = np.asarray(x, np.float32)
    rope = np.asarray(rope, np.float32)
    w_attn = np.asarray(w_attn, np.float32)
    w_proj = np.asarray(w_proj, np.float32)

    sin = rope[:, :, 0]                      # [L, 64]
    cos = rope[:, :, 1]
    cs = (np.concatenate([cos.T, cos.T], 0) * SCALE).astype(BF16)   # [128, L]
    ss = (np.concatenate([-sin.T, sin.T], 0) * SCALE).astype(BF16)

    p = np.arange(128)[:, None]
    f = np.arange(512)[None, :]
    masks = np.zeros((128, 4, 512), np.float32)
    for r in range(4):
        masks[:, r, :] = (r * 128 + p <= f).astype(np.float32)
    masks = masks.reshape(128, 4 * 512).astype(BF16)

    ones = np.ones((128, 128), np.float32).astype(BF16)
    perm = np.zeros((128, 128), np.float32)
    perm[(np.arange(128) + 64) % 128, np.arange(128)] = 1.0
    perm = perm.astype(BF16)

    xt_b = [np.ascontiguousarray(x[b].T).astype(BF16) for b in range(B)]

    wqk_g, wv_g, wout_g = {}, {}, {}
    for g in range(G):
        heads = [g * HPG + hl for hl in range(HPG)]
        wq = [np.ascontiguousarray(
                 w_attn[h * 128:(h + 1) * 128, :][_PERM_IDX, :].T) for h in heads]
        wk = [np.ascontiguousarray(
                 w_attn[E + h * 128:E + (h + 1) * 128, :][_PERM_IDX, :].T)
              for h in heads]
        wqk_g[g] = np.concatenate(wq + wk, axis=1).astype(BF16)        # [E, 1024]
        wv_g[g] = np.concatenate(
            [w_attn[2 * E + h * 128:2 * E + (h + 1) * 128, :].T for h in heads],
            axis=1).astype(BF16)                                        # [E, 512]
        wout_g[g] = np.ascontiguousarray(
            w_proj[:, g * 512:(g + 1) * 512].T).astype(BF16)            # [512, E]

    in_maps = []
    for c in range(NCORES):
        b, g = divmod(c, G)
        in_maps.append({
            "xt": xt_b[b],
            "wqk": wqk_g[g],
            "wv": wv_g[g],
            "wout": wout_g[g],
            "cs": cs,
            "ss": ss,
            "masks": masks,
            "ones": ones,
            "perm": perm,
        })
    return in_maps


def assemble_output(results):
    out = np.zeros((B, L, E), np.float32)
    for c in range(NCORES):
        b, g = divmod(c, G)
        out[b] += results[c]["out"].T
    return out


_NC = None


def get_nc():
    global _NC
    if _NC is None:
        _NC = build_nc()
    return _NC


def run(x, rope, w_attn, w_proj, trace=False, tmpdir=None):
    nc = get_nc()
    in_maps = prep_in_maps(x, rope, w_attn, w_proj)
    kwargs = {}
    if trace:
        import sys
        import types
        from concourse import bass_utils as _bu
        try:
            from trn_agent_boot.trn_boot import _ntff_profile_via_ctypes
            hook = _ntff_profile_via_ctypes("/opt/axon/libaxon_pjrt.so")
            mod = types.ModuleType("antenv.axon_hooks")
            mod.get_axon_ntff_profile_hook = lambda: hook
            sys.modules["antenv.axon_hooks"] = mod
            _bu.upload_artifacts = lambda dd: dd
        except Exception as e:  # pragma: no cover
            print("trace hook unavailable:", e)
        kwargs = dict(trace=True, tmpdir=tmpdir)
    res = run_bass_kernel_spmd(nc, in_maps, core_ids=list(range(NCORES)), **kwargs)
    return assemble_output(res.results), res


def kernel(x, rope, w_attn, w_proj):
    out, _ = run(x, rope, w_attn, w_proj, trace=False)
    return out
